# revision 24
# baseline (speedup 1.0000x reference)
"""Trainium2 Bass kernel for octonion causal self-attention (bf16 redesign).

Sharding: 8 cores = 4 batches x 2 head-groups. Core c handles batch b=c//2 and
head-group g=c%2 (octonion output components 4g..4g+3 = heads 8g..8g+7).
Each core computes q/k/v projections for its components from the full x[b],
RoPE, causal attention for its 8 heads, the octonion head-mixer for its group,
and a partial wo projection (its 4 input components, all 2048 output channels).
The host sums the two bf16 partials per batch and transposes. No collectives.

All matmuls run in bf16 (same PE rate as f32r on trn2, but half the DMA/SBUF
traffic and 2x DVE throughput). PSUM accumulation stays f32. V tiles stay in
SBUF (no DRAM spill). Causal structure is exploited: S matmuls, exp and PV
are trimmed to the lower triangle at 128-column granularity; the causal mask
is a single 128x128 identity-stationary matmul accumulated onto the diagonal
block of the score PSUM. Probs transposes are regular 128-free matmuls
(stationary = exp'd scores) whose moving operand is diag(1/l), folding the
softmax normalization into the transpose for free. The head-mixer is fused
into wo on the host (two consecutive per-token linear maps). Attention runs
as one 16-step (t-chunk, head) pipeline, 2 deep to hide exp latency, with
the first chunk's fused wo emitted into later steps as its heads complete.
Host-side layouts are partition-major so x/wv/wm/wo each load in 1-2 large
contiguous DMAs; pools whose DMAs are urgent at a repeat boundary (x, wqk)
persist across repeats so their loads overlap the previous body.
"""

import math
import os
from contextlib import ExitStack

import numpy as np

B, T, C, H, D = 4, 1024, 2048, 16, 128
C8 = C // 8  # 256
NCORES = 8
P = 128
NEGM = -30000.0


# ---------------- octonion tables (matches reference) ----------------
def _cd_conj(a):
    n = a.shape[0]
    if n == 1:
        return a
    h = n // 2
    return np.concatenate([_cd_conj(a[:h]), -a[h:]])


def _cd_mul(a, b):
    n = a.shape[0]
    if n == 1:
        return a * b
    h = n // 2
    a1, a2 = a[:h], a[h:]
    c1, c2 = b[:h], b[h:]
    return np.concatenate(
        [
            _cd_mul(a1, c1) - _cd_mul(_cd_conj(c2), a2),
            _cd_mul(c2, a1) + _cd_mul(a2, _cd_conj(c1)),
        ]
    )


def _octonion_tables():
    signs = np.zeros((8, 8), dtype=np.float32)
    widx = np.zeros((8, 8), dtype=np.int32)
    for i in range(8):
        for j in range(8):
            ei = np.zeros(8)
            ei[i] = 1.0
            ej = np.zeros(8)
            ej[j] = 1.0
            p = _cd_mul(ei, ej)
            k = int(np.argmax(np.abs(p)))
            signs[i, j] = np.sign(p[k])
            widx[i, j] = k
    return signs, widx


SIGNS, WIDX = _octonion_tables()

_EVENS_FIRST = np.concatenate([np.arange(0, D, 2), np.arange(1, D, 2)])


def _bf16(a):
    import ml_dtypes

    return np.asarray(a, dtype=np.float32).astype(ml_dtypes.bfloat16)


def _ternary_quantize(W: np.ndarray) -> np.ndarray:
    """Replicates reference ternary_ste forward pass bit-exactly (jnp on CPU)."""
    import jax
    import jax.numpy as jnp

    with jax.default_device(jax.devices("cpu")[0]):
        Wj = jnp.asarray(W)
        s = jnp.mean(jnp.abs(Wj), axis=(-2, -1), keepdims=True) + 1e-8
        Wq = jnp.clip(jnp.round(Wj / s), -1.0, 1.0) * s
        return np.asarray(Wq)


def _signed_full(Wq: np.ndarray, i: int) -> np.ndarray:
    """[2048, 256] block column for octonion output component i:
    rows j*256:(j+1)*256 = SIGNS[i,j] * Wq[i^j]."""
    out = np.empty((C, C8), dtype=np.float32)
    for j in range(8):
        out[j * C8 : (j + 1) * C8, :] = SIGNS[i, j] * Wq[i ^ j]
    return out


def _prep_core_inputs(inputs: dict, b: int, g: int, wq_q, wk_q, wv_q, wo_q):
    x = inputs["x"]
    fc, fs = inputs["freqs_cos"], inputs["freqs_sin"]
    mixer_W, mixer_beta = inputs["mixer_W"], inputs["mixer_beta"]

    m = {}
    # x transposed, partition-major: [p, ct, t] bf16 (one big DMA)
    m["xT"] = _bf16(np.ascontiguousarray(x[b].T).reshape(16, P, T).transpose(1, 0, 2))

    # q/k weights: [qk, li, dh, c_p, ct, d], interleaved rope layout
    wqk = np.empty((2, 4, 2, P, 16, P), dtype=np.float32)
    qscale = np.float32(1.0 / math.sqrt(D))
    for qk, Wq in enumerate((wq_q, wk_q)):
        for li in range(4):
            i = 4 * g + li
            Bf = _signed_full(Wq, i)  # [2048, 256]
            if qk == 0:
                Bf = Bf * qscale
            for dh in range(2):
                Bh = Bf[:, dh * D : (dh + 1) * D]  # [2048, 128]
                wqk[qk, li, dh] = Bh[:, _EVENS_FIRST].reshape(16, P, P).transpose(
                    1, 0, 2
                )
    m["wqk"] = _bf16(wqk)

    # v weights: [lp, ct, c_p, dcol] with dcol = 2 comps x 256, natural order
    wv = np.empty((2, 16, P, 512), dtype=np.float32)
    for lp in range(2):
        B2 = np.concatenate(
            [_signed_full(wv_q, 4 * g + 2 * lp + u) for u in range(2)], axis=1
        )  # [2048, 512]
        wv[lp] = B2.reshape(16, P, 512)
    m["wv"] = _bf16(wv.transpose(2, 0, 1, 3))  # [p, lp, ct, d]

    # wo with the head-mixer fused in: both are per-token linear maps on the
    # local 1024-dim feature space (z = M y, out = Wo^T z), so Wo_fused =
    # M^T Wo folds the mixer away entirely.  Local z/y channel = head*128+dim.
    wo = np.empty((16, P, 8, P), dtype=np.float32)
    for ft in range(16):
        i_o, fh = ft // 2, ft % 2
        for kt in range(8):
            j = 4 * g + kt // 2
            dloc = kt % 2
            blk = SIGNS[i_o, j] * wo_q[i_o ^ j]  # [256, 256]
            wo[ft, :, kt, :] = blk[dloc * P : (dloc + 1) * P, fh * P : (fh + 1) * P]
    A = wo.transpose(2, 1, 0, 3).reshape(1024, 2048)  # [(kt,p) z-chan, f]
    M = np.zeros((1024, 1024), dtype=np.float32)  # [z (i,e), y (j,d)]
    for i in range(8):
        for j in range(8):
            M[i * P : (i + 1) * P, j * P : (j + 1) * P] = (
                SIGNS[i, j] * mixer_W[i ^ j] * mixer_beta[None, :]
            ).T
    Af = M.T @ A  # [(j,d) y-chan, f]
    m["wo"] = _bf16(
        Af.reshape(8, P, 16, P).transpose(1, 2, 0, 3)
    )  # [p, ft, kt, f]

    # evens-first rope layout: rows 0..63 = even dims, 64..127 = odd dims.
    # rope(q')[p] = q'[p]*cosd[p] + q'[p xor 64]*sind[p]; swap = half exchange
    cosP = np.ascontiguousarray(fc.T)  # [64, 1024]
    sinP = np.ascontiguousarray(fs.T)
    m["cosd"] = _bf16(np.concatenate([cosP, cosP], axis=0))
    m["sind"] = _bf16(np.concatenate([-sinP, sinP], axis=0))
    m["ident"] = _bf16(np.eye(P, dtype=np.float32))

    # causal mask for the 128x128 diagonal block: allow col j <= row p
    pidx = np.arange(P)[:, None]
    jidx = np.arange(P)[None, :]
    m["tri"] = _bf16(np.where(jidx <= pidx, 0.0, NEGM).astype(np.float32))
    return m


# ---------------- device program ----------------
_NC_CACHE = {}


def _build_nc(repeat: int = 1, pp_bufs: int = 2, attn_depth: int = 2):
    key = (repeat, pp_bufs, attn_depth)
    if key in _NC_CACHE:
        return _NC_CACHE[key]

    import concourse.mybir as mybir
    import concourse.tile as tile
    from concourse import bacc

    dt = mybir.dt
    ALU = mybir.AluOpType
    AF = mybir.ActivationFunctionType
    f32, bf16 = dt.float32, dt.bfloat16

    nc = bacc.Bacc("TRN2", target_bir_lowering=False)

    xT = nc.declare_dram_parameter("xT", [P, 16, T], bf16, isOutput=False)
    wqk = nc.declare_dram_parameter("wqk", [2, 4, 2, P, 16, P], bf16, isOutput=False)
    wv = nc.declare_dram_parameter("wv", [P, 2, 16, 512], bf16, isOutput=False)
    wo = nc.declare_dram_parameter("wo", [P, 16, 8, P], bf16, isOutput=False)
    cosd = nc.declare_dram_parameter("cosd", [P, T], bf16, isOutput=False)
    sind = nc.declare_dram_parameter("sind", [P, T], bf16, isOutput=False)
    trip = nc.declare_dram_parameter("tri", [P, P], bf16, isOutput=False)
    identp = nc.declare_dram_parameter("ident", [P, P], bf16, isOutput=False)
    outT = nc.declare_dram_parameter("outT", [C, T], bf16, isOutput=True)

    with tile.TileContext(nc) as tc, ExitStack() as ctx:
        cst = ctx.enter_context(tc.tile_pool(name="cst", bufs=1))
        statp = ctx.enter_context(tc.tile_pool(name="statp", bufs=16))
        xp = ctx.enter_context(tc.tile_pool(name="xp", bufs=1))
        wqk_pool = ctx.enter_context(tc.tile_pool(name="wqkp", bufs=2))
        ropeA = ctx.enter_context(tc.tile_pool(name="ropeA", bufs=3))
        ropeB = ctx.enter_context(tc.tile_pool(name="ropeB", bufs=2))
        qks = ctx.enter_context(tc.tile_pool(name="qks", bufs=1))
        vsb = ctx.enter_context(tc.tile_pool(name="vsb", bufs=1))
        # PSUM pools: pst+psy always open (4 banks); pp (proj, 4 banks) and
        # pss (S, [128,1024] = 2 banks x 2 bufs) are phase-local.
        ps_big = ctx.enter_context(tc.tile_pool(name="psbig", bufs=2, space="PSUM"))
        ps_t = ctx.enter_context(tc.tile_pool(name="pst", bufs=2, space="PSUM"))
        ps_y = ctx.enter_context(tc.tile_pool(name="psy", bufs=2, space="PSUM"))

        # constants loaded once (gpsimd SWDGE queue)
        ident = cst.tile([P, P], bf16, tag="ident")
        nc.gpsimd.dma_start(ident[:], identp[:])
        cos_sb = cst.tile([P, T], bf16, tag="cos")
        nc.gpsimd.dma_start(cos_sb[:], cosd[:])
        sin_sb = cst.tile([P, T], bf16, tag="sin")
        nc.gpsimd.dma_start(sin_sb[:], sind[:])
        tri_sb = cst.tile([P, P], bf16, tag="tri")
        nc.gpsimd.dma_start(tri_sb[:], trip[:])

        for _rep in range(repeat):
            qT_h = [qks.tile([P, T], bf16, tag=f"qT{i}", name=f"qTh{i}") for i in range(8)]
            kT_h = [qks.tile([P, T], bf16, tag=f"kT{i}", name=f"kTh{i}") for i in range(8)]
            v_t = [
                [
                    vsb.tile([P, 2, 512], bf16, tag=f"v{lp}_{tp}", name=f"vt{lp}_{tp}")
                    for tp in range(4)
                ]
                for lp in range(2)
            ]
            wv_cm = tc.tile_pool(name="wvp", bufs=1)
            wv_pool = wv_cm.__enter__()


            # batched x + wv loads (2 big DMAs each, on separate queues)
            x_all = xp.tile([P, 16, T], bf16, tag="xall", name="xall")
            nc.sync.dma_start(x_all[:, 0:8, :], xT[:, 0:8, :])
            nc.sync.dma_start(x_all[:, 8:16, :], xT[:, 8:16, :])
            x_t = [x_all[:, ct, :] for ct in range(16)]
            wv_all = wv_pool.tile([P, 2, 16, 512], bf16, tag="wvall", name="wvall")
            nc.gpsimd.dma_start(wv_all[:, 0], wv[:, 0])
            nc.gpsimd.dma_start(wv_all[:, 1], wv[:, 1])
            wv_tiles = {(lp, ct): wv_all[:, lp, ct, :] for lp in range(2) for ct in range(16)}

            # ---- Q/K projections with fused RoPE (DMA half-swap,
            # pipelined one iteration) ----
            def emit_rope(dest, hh, qsb):
                qsw = ropeB.tile([P, T], bf16, tag="qsw")
                nc.sync.dma_start(qsw[0:64, :], qsb[64:128, :])
                nc.scalar.dma_start(qsw[64:128, :], qsb[0:64, :])
                t1 = ropeB.tile([P, T], bf16, tag="t1")
                t2 = ropeB.tile([P, T], bf16, tag="t2")
                nc.vector.tensor_tensor(t1[:], qsb[:], cos_sb[:], ALU.mult)
                nc.vector.tensor_tensor(t2[:], qsw[:], sin_sb[:], ALU.mult)
                nc.vector.tensor_tensor(dest[hh][:], t1[:], t2[:], ALU.add)

            rope_pending = None
            for qk, dest_h in ((0, qT_h), (1, kT_h)):
                for li in range(4):
                    for dh in range(2):
                        hh = li * 2 + dh
                        wt = wqk_pool.tile([P, 16, P], bf16, tag="wqk")
                        eng = nc.sync if (hh % 2 == 0) else nc.gpsimd
                        eng.dma_start(wt[:], wqk[qk, li, dh])
                        pps = ps_big.tile([P, T], f32, tag="big")
                        for ct in range(16):
                            for tci in range(2):
                                nc.tensor.matmul(
                                    pps[:, tci * 512 : (tci + 1) * 512],
                                    wt[:, ct, :],
                                    x_all[:, ct, tci * 512 : (tci + 1) * 512],
                                    start=(ct == 0),
                                    stop=(ct == 15),
                                )
                        qsb = ropeA.tile([P, T], bf16, tag="qsb")
                        nc.vector.tensor_copy(out=qsb[:], in_=pps[:])
                        if rope_pending is not None:
                            emit_rope(*rope_pending)
                        rope_pending = (dest_h, hh, qsb)
            emit_rope(*rope_pending)

            # ---- V projection (stays in SBUF), token tiles paired so each
            # wide PSUM tile drains in one copy ----
            for lp in range(2):
                for tp in range(4):
                    vps = ps_big.tile([P, T], f32, tag="big", name=f"vp{lp}_{tp}")
                    for sub in range(2):
                        tt = 2 * tp + sub
                        for ct in range(16):
                            nc.tensor.matmul(
                                vps[:, sub * 512 : (sub + 1) * 512],
                                x_all[:, ct, tt * P : (tt + 1) * P],
                                wv_all[:, lp, ct, :],
                                start=(ct == 0),
                                stop=(ct == 15),
                            )
                    nc.vector.tensor_copy(out=v_t[lp][tp][:], in_=vps[:])

            wv_cm.__exit__(None, None, None)

            # attention-phase pools
            psb_cm = tc.tile_pool(name="psb", bufs=13)
            psb = psb_cm.__enter__()
            diag_cm = tc.tile_pool(name="diagp", bufs=13)
            diagp = diag_cm.__enter__()
            pt_cm = tc.tile_pool(name="ptsb", bufs=10)
            ptsb = pt_cm.__enter__()
            y_cm = tc.tile_pool(name="ysb", bufs=2)
            yp = y_cm.__enter__()
            wo_cm = tc.tile_pool(name="wop", bufs=1)
            wop = wo_cm.__enter__()
            out_cm = tc.tile_pool(name="outp", bufs=2)
            outp = out_cm.__enter__()

            wo_all = wop.tile([P, 16, 8, P], bf16, tag="woall", name="woall")
            nc.gpsimd.dma_start(wo_all[:, 0:8], wo[:, 0:8])
            nc.gpsimd.dma_start(wo_all[:, 8:16], wo[:, 8:16])
            wo_t = [wo_all[:, ft] for ft in range(16)]

            def emit_wo_ft(ft, z_src, tsl_prev, osb_box):
                if ft % 4 == 0:
                    osb_box[0] = outp.tile(
                        [P, 4, 512], bf16, tag="osb", name=f"osbd{ft}_{_rep}"
                    )
                osb = osb_box[0]
                ops = ps_t.tile([P, 512], f32, tag="tp")
                for kt in range(8):
                    nc.tensor.matmul(
                        ops[:],
                        wo_t[ft][:, kt, :],
                        z_src[:, kt, :],
                        start=(kt == 0),
                        stop=(kt == 7),
                    )
                nc.vector.tensor_copy(out=osb[:, ft % 4, :], in_=ops[:])
                if ft % 4 == 3:
                    f0 = ft - 3
                    nc.scalar.dma_start(
                        outT[f0 * P : (f0 + 4) * P, tsl_prev].rearrange(
                            "(f p) t -> p f t", p=P
                        ),
                        osb[:],
                    )

            def emit_S(h, qc):
                """S matmuls + diag mask + exp + recip + diag(r) for 4 q-blocks."""
                Ps_list = {}
                diag_list = {}
                lt_all = statp.tile([P, 4], f32, tag="l")
                rec_all = statp.tile([P, 4], f32, tag="r")
                for qt in range(4 * qc, 4 * qc + 4):
                    wq_w = (qt + 1) * P
                    Ps = psb.tile([P, T], bf16, tag="P", name=f"Ps{qc}_{h}_{qt}")
                    Ps_list[qt] = Ps
                    sps = ps_big.tile([P, T], f32, tag="big", name=f"sps{qc}_{h}_{qt}")
                    for chi in range((wq_w + 511) // 512):
                        w = min(512, wq_w - chi * 512)
                        nc.tensor.matmul(
                            sps[:, chi * 512 : chi * 512 + w],
                            qT_h[h][:, qt * P : (qt + 1) * P],
                            kT_h[h][:, chi * 512 : chi * 512 + w],
                            start=True,
                            stop=False,
                            skip_group_check=True,
                        )
                    nc.tensor.matmul(
                        sps[:, wq_w - P : wq_w],
                        ident[:],
                        tri_sb[:],
                        start=False,
                        stop=True,
                        skip_group_check=True,
                    )
                    nc.scalar.activation(
                        Ps[:, :wq_w],
                        sps[:, :wq_w],
                        AF.Exp,
                        accum_out=lt_all[:, qt % 4 : qt % 4 + 1],
                    )
                nc.vector.reciprocal(rec_all[:], lt_all[:])
                for qt in range(4 * qc, 4 * qc + 4):
                    dg = diagp.tile([P, P], bf16, tag="diag", name=f"dg{qc}_{h}_{qt}")
                    nc.vector.tensor_scalar(
                        dg[:], ident[:], rec_all[:, qt % 4 : qt % 4 + 1], None,
                        op0=ALU.mult,
                    )
                    diag_list[qt] = dg
                return Ps_list, diag_list

            def emit_TPV(h, qc, Ps_list, diag_list, y_sb):
                """Normalizing transposes (regular matmuls vs diag(1/l)) + PV."""
                nkt = 4 * (qc + 1)
                pts = []
                for kt in range(nkt):
                    qt0 = max(kt, 4 * qc)
                    off = (qt0 - 4 * qc) * P
                    ptps = ps_t.tile([P, 512], f32, tag="tp")
                    for qt in range(qt0, 4 * qc + 4):
                        cl = (qt % 4) * P
                        nc.tensor.matmul(
                            ptps[:, cl : cl + P],
                            Ps_list[qt][:, kt * P : (kt + 1) * P],
                            diag_list[qt][:],
                            start=True,
                            stop=True,
                        )
                    pt_sb = ptsb.tile([P, 512], bf16, tag="PT", name=f"PT{qc}_{h}_{kt}")
                    nc.vector.tensor_copy(out=pt_sb[:, off:], in_=ptps[:, off:])
                    pts.append((pt_sb, off))
                yps = ps_y.tile([P, 512], f32, tag="y")
                lp, dcol = h // 4, (h % 4) * P
                for kt in range(nkt):
                    pt_sb, off = pts[kt]
                    nc.tensor.matmul(
                        yps[:, off:],
                        v_t[lp][kt // 2][:, kt % 2, dcol : dcol + P],
                        pt_sb[:, off:],
                        start=(kt == 0),
                        stop=(kt == nkt - 1),
                        skip_group_check=True,
                    )
                nc.vector.tensor_copy(out=y_sb[:, h, :], in_=yps[:])

            # ---- one 16-step attention pipeline over (qc, h); qc0's wo
            # chunks flow in as soon as its heads complete ----
            y_sbs = {
                qc: yp.tile([P, 8, 512], bf16, tag="y", name=f"ysb{qc}")
                for qc in range(2)
            }
            tsls = {qc: slice(qc * 512, (qc + 1) * 512) for qc in range(2)}
            steps = [(qc, h) for qc in range(2) for h in range(8)]
            pending = []
            wo_queue = []
            osb_box = [None]
            for qc, h in steps:
                pending.append(((qc, h), emit_S(h, qc)))
                for _ in range(3):
                    if wo_queue:
                        emit_wo_ft(wo_queue.pop(0), y_sbs[0], tsls[0], osb_box)
                if len(pending) > attn_depth:
                    (pqc, ph), cur = pending.pop(0)
                    emit_TPV(ph, pqc, cur[0], cur[1], y_sbs[pqc])
                    if (pqc, ph) == (0, 7):
                        wo_queue = list(range(16))
            for (pqc, ph), cur in pending:
                emit_TPV(ph, pqc, cur[0], cur[1], y_sbs[pqc])
            for ft in wo_queue:
                emit_wo_ft(ft, y_sbs[0], tsls[0], osb_box)
            box = [None]
            for ft in range(16):
                emit_wo_ft(ft, y_sbs[1], tsls[1], box)

            out_cm.__exit__(None, None, None)
            wo_cm.__exit__(None, None, None)
            y_cm.__exit__(None, None, None)
            pt_cm.__exit__(None, None, None)
            diag_cm.__exit__(None, None, None)
            psb_cm.__exit__(None, None, None)

    nc.finalize()
    _NC_CACHE[key] = nc
    return nc


def _run(inputs: dict, trace: bool = False):
    from concourse.bass_utils import run_bass_kernel_spmd

    wq_q = _ternary_quantize(np.asarray(inputs["wq"], dtype=np.float32))
    wk_q = _ternary_quantize(np.asarray(inputs["wk"], dtype=np.float32))
    wv_q = _ternary_quantize(np.asarray(inputs["wv"], dtype=np.float32))
    wo_q = _ternary_quantize(np.asarray(inputs["wo"], dtype=np.float32))

    in_maps = []
    for c in range(NCORES):
        b, g = c // 2, c % 2
        in_maps.append(_prep_core_inputs(inputs, b, g, wq_q, wk_q, wv_q, wo_q))

    nc = _build_nc()
    res = run_bass_kernel_spmd(nc, in_maps, list(range(NCORES)), trace=trace)

    out = np.empty((B, T, C), dtype=np.float32)
    for b in range(B):
        acc = np.asarray(res.results[2 * b]["outT"]).astype(np.float32) + np.asarray(
            res.results[2 * b + 1]["outT"]
        ).astype(np.float32)
        out[b] = acc.T
    return out, res


def kernel(**inputs) -> np.ndarray:
    out, _ = _run(inputs, trace=False)
    return out
